# revision 8
# baseline (speedup 1.0000x reference)
"""Trainium2 Bass kernel for nn_DeConvAfterDownSampling.

Math (from the reference): with s[n] = sum_w x[b,c,h,w] flattened over
n = (b,c,h), Wf = W.reshape(F, P):

    out[0, f, n, p] = relu(s[n] * Wf[f, p] + b[f])      # (1, F, N, P)

N = 8*64*64 = 32768, F = 64, P = 25.  Output is ~210 MB fp32 while inputs
are ~8 MB, so the kernel is bound by the output HBM write.

Sharding: data-parallel over n across 8 cores (N_LOCAL = 4096 per core);
W and b replicated; no cross-core communication.

Per-core plan (partitions = (h, f) with h in {0,1} stacking two n-halves
so all 128 partitions are used):
  1. One DMA loads x (4096, 64) into SBUF as (128, 32, 64), partition
     i <- row 128*t + i.
  2. PE transposes each (128 n, 64 w) block -> (64 w, 128 n) in PSUM;
     copies assemble xT (128=(h,w), 512 n) in SBUF.
  3. One K=128 matmul with a constant block-diagonal ones matrix E
     (E[(h',w),(h,f)] = (h==h')) reduces over w and broadcasts:
     s_bcast[(h,f), j] = s[tile_base + 512h + j] for every f.
  4. For each p in 0..24 one elementwise op computes
     relu(W[f,p] * s + b[f]) with W[:,p] as per-partition scale and b as
     per-partition bias, writing the (stride 25) p-slice of the output
     tile.  Ops are split across ScalarE (activation, reads PSUM) and
     VectorE (tensor_scalar mult+max, reads an SBUF copy).
  5. One ~6.5 MB DMA per tile writes the (128, 512, 25) tile to HBM; the
     per-partition free layout (n-major, p-minor) is exactly contiguous
     HBM order, so each partition is a single 51.2 KB contiguous chunk.
"""

import numpy as np

import concourse.bass as bass
import concourse.mybir as mybir
from concourse import bacc, masks, tile
from concourse.bass_utils import run_bass_kernel_spmd

F32 = mybir.dt.float32

N_CORES = 8
B, C, H, WDIM = 8, 64, 64, 64
F, P = 64, 25
N_TOTAL = B * C * H          # 32768
N_LOCAL = N_TOTAL // N_CORES  # 4096
TILE_N = 1024                 # n per compute tile
HALF = TILE_N // 2            # 512 n per partition-half
N_TILES = N_LOCAL // TILE_N   # 4
NPART = 128

# Engine split for the 25 per-p elementwise ops (b == 0 fast path).
SCALAR_PS = set(range(13))    # p handled by ScalarE activation
# remaining p handled by VectorE tensor_scalar(mult, max 0)


def build_bass(with_bias: bool) -> bass.Bass:
    nc = bacc.Bacc(None)

    x_d = nc.dram_tensor("x", (N_LOCAL, WDIM), F32, kind="ExternalInput")
    w_d = nc.dram_tensor("W", (F, P), F32, kind="ExternalInput")
    b_d = nc.dram_tensor("b", (F, 1), F32, kind="ExternalInput")
    o_d = nc.dram_tensor("out", (F, N_LOCAL, P), F32, kind="ExternalOutput")

    with tile.TileContext(nc) as tc:
        with (
            tc.tile_pool(name="const", bufs=1) as constp,
            tc.tile_pool(name="xin", bufs=1) as xinp,
            tc.tile_pool(name="work", bufs=2) as workp,
            tc.tile_pool(name="outp", bufs=2) as outp,
            tc.tile_pool(name="psum", bufs=2, space="PSUM") as psump,
        ):
            # --- constants ---
            ident = constp.tile([NPART, NPART], F32)
            masks.make_identity(nc, ident[:])

            # Block-diagonal ones: E[k, i] = 1 iff k//64 == i//64.
            e_mat = constp.tile([NPART, NPART], F32)
            nc.gpsimd.memset(e_mat[:], 0.0)
            nc.gpsimd.memset(e_mat[0:64, 0:64], 1.0)
            nc.gpsimd.memset(e_mat[64:128, 64:128], 1.0)

            # W columns and bias replicated on both partition halves.
            wcols = constp.tile([NPART, P], F32)
            nc.sync.dma_start(wcols[0:64, :], w_d[:, :])
            nc.sync.dma_start(wcols[64:128, :], w_d[:, :])
            bcol = constp.tile([NPART, 1], F32)
            nc.sync.dma_start(bcol[0:64, :], b_d[:, :])
            nc.sync.dma_start(bcol[64:128, :], b_d[:, :])

            # --- load x: partition i holds rows {128t + i} ---
            x_sb = xinp.tile([NPART, N_LOCAL // NPART, WDIM], F32)
            nc.sync.dma_start(x_sb[:], x_d[:, :].rearrange("(t i) w -> i t w", i=NPART))

            out_r = o_d[:, :, :].rearrange(
                "f (u h j) p -> u h f j p", h=2, j=HALF
            )  # (N_TILES, 2, 64, HALF, P)

            for u in range(N_TILES):
                # --- transpose 8 n-blocks of 128 into (h, w) layout ---
                xt_ps = [
                    psump.tile([64, 4, NPART], F32, name=f"xtp{h}", tag=f"xtp{h}")
                    for h in range(2)
                ]
                for t in range(8):
                    h, slot = t // 4, t % 4
                    nc.tensor.transpose(
                        xt_ps[h][:, slot, :], x_sb[:, 8 * u + t, :], ident[:]
                    )
                xt_sb = workp.tile([NPART, 4, NPART], F32, tag="xt_sb")
                nc.vector.tensor_copy(xt_sb[0:64], xt_ps[0][:])
                nc.vector.tensor_copy(xt_sb[64:128], xt_ps[1][:])

                # --- s broadcast: one matmul, K=128 ---
                s_ps = psump.tile([NPART, HALF], F32, tag="s_ps")
                nc.tensor.matmul(s_ps[:], e_mat[:], xt_sb[:])

                s_sb = workp.tile([NPART, HALF], F32, tag="s_sb")
                nc.vector.tensor_copy(s_sb[:], s_ps[:])

                # --- 25 per-p elementwise ops ---
                out_t = outp.tile([NPART, HALF, P], F32, tag="out_t")
                for p in range(P):
                    if with_bias or p in SCALAR_PS:
                        nc.scalar.activation(
                            out_t[:, :, p],
                            s_ps[:],
                            mybir.ActivationFunctionType.Relu,
                            bias=bcol[:, 0:1],
                            scale=wcols[:, p : p + 1],
                        )
                    else:
                        nc.vector.tensor_scalar(
                            out_t[:, :, p],
                            s_sb[:],
                            wcols[:, p : p + 1],
                            0.0,
                            mybir.AluOpType.mult,
                            mybir.AluOpType.max,
                        )

                nc.sync.dma_start(out_r[u], out_t[:])

    nc.compile()
    return nc


_CACHE: dict[bool, bass.Bass] = {}


def _get_bass(with_bias: bool) -> bass.Bass:
    if with_bias not in _CACHE:
        _CACHE[with_bias] = build_bass(with_bias)
    return _CACHE[with_bias]


last_exec_time_ns = None
last_profile = None


def kernel(x, W, b, trace=False, **run_kwargs):
    global last_exec_time_ns, last_profile
    x = np.ascontiguousarray(np.asarray(x, dtype=np.float32)).reshape(N_TOTAL, WDIM)
    wf = np.ascontiguousarray(np.asarray(W, dtype=np.float32)).reshape(F, P)
    bf = np.ascontiguousarray(np.asarray(b, dtype=np.float32)).reshape(F, 1)

    nc = _get_bass(bool(np.any(bf)))

    in_maps = [
        {
            "x": x[m * N_LOCAL : (m + 1) * N_LOCAL],
            "W": wf,
            "b": bf,
        }
        for m in range(N_CORES)
    ]
    res = run_bass_kernel_spmd(
        nc, in_maps, core_ids=list(range(N_CORES)), trace=trace, **run_kwargs
    )
    last_exec_time_ns = res.exec_time_ns
    last_profile = res.profile_json
    outs = [np.asarray(res.results[m]["out"]) for m in range(N_CORES)]
    full = np.concatenate(outs, axis=1)  # (F, N_TOTAL, P)
    return full[None]
